# revision 4
# baseline (speedup 1.0000x reference)
"""BiLSTM tagger kernel for 8 Trainium2 NeuronCores.

Model (per reference): x = emb[tokens]; h_f = LSTM_f(x); h_b = LSTM_b(rev(x));
probs = softmax([h_f, h_b] @ Wd + bd).

Sharding: data-parallel over batch. Each of the 8 cores handles 32 sequences
and runs BOTH directions for them, so no cross-core communication is needed;
the host shards tokens and concatenates outputs.

Per-core layout ("transposed" LSTM): everything keeps the feature dim on SBUF
partitions and the 32 sequences on the free dim.  Token slot s = seq + 32*t.
 - gather: emb rows -> x_raw [128 part, slots/128, 256]  (indirect DMA)
 - PE-transpose -> xT [128 (E-slice), kt, slots] bf16
 - projection:  xzT[dir] [128 (4H-slice), m, slots] = W^T x + b   (bf16, bias
   folded, gates reordered host-side to [i, f, o, g] so sigmoid gates are
   contiguous)
 - recurrence (per direction, 128 steps): zT = U^T h in PSUM (16 matmuls,
   N=32), += xzT_t (DVE), sigmoid/tanh (ACT), cell update (DVE, fp32 cell),
   h written straight in matmul-rhs layout (no per-step transpose).
 - dense+softmax: logits accumulated incrementally per 8-step chunk from both
   directions, then bias + exp + normalize at the end.

Weights are marshalled host-side into the exact SBUF tile layouts (k-tile on
partitions) and cast to bf16; cell state and all accumulations stay fp32.
"""

import sys

import numpy as np

if "/opt/trn_rl_repo" not in sys.path:
    sys.path.insert(0, "/opt/trn_rl_repo")

V, E, T, H, NTAGS, B = 50000, 256, 128, 256, 17, 256
NCORES = 8
BS = B // NCORES            # sequences per core
P = 128
KT = E // P                 # 2 k-tiles for E and H
M8 = (4 * H) // P           # 8 m-tiles over the gate dim
# The SWDGE indirect-DMA (gather) path is unreliable in this environment
# (works after boot, breaks persistently after any device fault), so the
# embedding rows are gathered host-side into the slot layout and streamed
# to the device as a regular input.  Device work is otherwise identical.
USE_HOST_GATHER = True

_CACHE = {}


def _legalize_waits(nc):
    """TRN2 hw instructions have one semaphore-wait slot; Tile can attach
    several.  Split extras onto same-engine NOPs placed just before."""
    import concourse.mybir as mybir

    for _, bbb in nc.bb_map.items():
        bb = bbb.bb
        new = []
        for inst in bb.instructions:
            si = inst.sync_info
            waits = list(si.on_wait) if (si and si.on_wait) else []
            if len(waits) > 1:
                for k, w in enumerate(waits[:-1]):
                    nop = mybir.InstNoOp(
                        name=f"{inst.name}_lw{k}",
                        engine=inst.engine,
                        sync_info=mybir.SyncInfo(on_wait=[w], on_update=[]),
                        bass_nofuse=True,
                    )
                    nc.register_instruction(nop)
                    new.append(nop)
                inst.sync_info = mybir.SyncInfo(
                    on_wait=[waits[-1]],
                    on_update=list(si.on_update) if si.on_update else [],
                )
            new.append(inst)
        bb.instructions = new


def build_program(t_len=T, vocab=V):
    """Build the per-core SPMD program.  t_len must be a multiple of 16."""
    from contextlib import ExitStack

    import concourse.bass as bass
    import concourse.mybir as mybir
    import concourse.tile as tile
    from concourse.masks import make_identity

    f32 = mybir.dt.float32
    bf16 = mybir.dt.bfloat16
    SIG = mybir.ActivationFunctionType.Sigmoid
    TANH = mybir.ActivationFunctionType.Tanh
    EXP = mybir.ActivationFunctionType.Exp
    MUL = mybir.AluOpType.mult
    ADD = mybir.AluOpType.add

    SLOTS = BS * t_len
    JT = SLOTS // P             # 128-slot tiles (= t_len/4)
    NCH = t_len // 16           # projection chunks of 512 slots
    NRCH = t_len // 8           # recurrence chunks of 8 steps

    nc = bass.Bass("TRN2", target_bir_lowering=False, debug=False)

    if USE_HOST_GATHER:
        xg = nc.dram_tensor("xg", [P, JT, E], f32, kind="ExternalInput")
    else:
        emb = nc.dram_tensor("emb", [vocab, E], f32, kind="ExternalInput")
        idx = nc.dram_tensor("idx", [P, JT], mybir.dt.int32, kind="ExternalInput")
    w_in = {d: nc.dram_tensor(f"w_{d}", [P, KT, M8, P], bf16, kind="ExternalInput")
            for d in "fb"}
    u_in = {d: nc.dram_tensor(f"u_{d}", [P, KT, M8, P], bf16, kind="ExternalInput")
            for d in "fb"}
    b_in = {d: nc.dram_tensor(f"b_{d}", [P, M8], f32, kind="ExternalInput")
            for d in "fb"}
    wd_in = nc.dram_tensor("wd", [P, 2 * KT, NTAGS], bf16, kind="ExternalInput")
    bd_in = nc.dram_tensor("bd", [P, 8 * NTAGS], f32, kind="ExternalInput")
    out = nc.dram_tensor("out", [P, JT, NTAGS], f32, kind="ExternalOutput")

    with tile.TileContext(nc) as tc, ExitStack() as ctx:
        cpool = ctx.enter_context(tc.tile_pool(name="const", bufs=1))
        xzpool = ctx.enter_context(tc.tile_pool(name="xz", bufs=1))
        xtpool = ctx.enter_context(tc.tile_pool(name="xt", bufs=1))
        xrpool = ctx.enter_context(tc.tile_pool(name="xr", bufs=2))
        gpool = ctx.enter_context(tc.tile_pool(name="g", bufs=2))
        hpool = ctx.enter_context(tc.tile_pool(name="h", bufs=2))
        spool = ctx.enter_context(tc.tile_pool(name="s", bufs=1))
        opool = ctx.enter_context(tc.tile_pool(name="o", bufs=2))
        tppool = ctx.enter_context(tc.tile_pool(name="tp", bufs=1, space="PSUM"))
        scpool = ctx.enter_context(tc.tile_pool(name="sc", bufs=1, space="PSUM"))
        prpool = ctx.enter_context(tc.tile_pool(name="pr", bufs=2, space="PSUM"))
        zpool = ctx.enter_context(tc.tile_pool(name="z", bufs=1, space="PSUM"))
        dpool = ctx.enter_context(tc.tile_pool(name="d", bufs=1, space="PSUM"))

        # ---- constant loads ----
        if not USE_HOST_GATHER:
            idx_sb = cpool.tile([P, JT], mybir.dt.int32)
            nc.sync.dma_start(idx_sb[:], idx[:])
        ident = cpool.tile([P, P], f32)
        make_identity(nc, ident[:])
        w_sb, u_sb, b_sb = {}, {}, {}
        for d in "fb":
            w_sb[d] = cpool.tile([P, KT, M8, P], bf16, tag=f"w{d}", name=f"wsb{d}")
            nc.sync.dma_start(w_sb[d][:], w_in[d][:])
            u_sb[d] = cpool.tile([P, KT, M8, P], bf16, tag=f"u{d}", name=f"usb{d}")
            nc.sync.dma_start(u_sb[d][:], u_in[d][:])
            b_sb[d] = cpool.tile([P, M8], f32, tag=f"b{d}", name=f"bsb{d}")
            nc.sync.dma_start(b_sb[d][:], b_in[d][:])
        wd_sb = cpool.tile([P, 2 * KT, NTAGS], bf16)
        nc.sync.dma_start(wd_sb[:], wd_in[:])
        bd_sb = cpool.tile([P, 8, NTAGS], f32)
        nc.sync.dma_start(bd_sb[:], bd_in[:])

        xzT = {d: xzpool.tile([P, M8, SLOTS], bf16, tag=f"xz{d}", name=f"xzT{d}") for d in "fb"}
        xT = xtpool.tile([P, KT, SLOTS], bf16)

        # PE-only scratch: absorbs cross-engine waits so transpose matmuls
        # (single hw wait slot) never need two.
        scr = scpool.tile([32, 32], f32)
        nc.tensor.transpose(out=scr[:], in_=ident[0:32, 0:32],
                            identity=ident[0:32, 0:32])

        # ---- gather + transpose + projection, chunk-pipelined ----
        # fwd consumes slots ascending, bwd descending: alternate chunk order.
        order = []
        lo, hi = 0, NCH - 1
        while lo <= hi:
            order.append(lo)
            if hi != lo:
                order.append(hi)
            lo, hi = lo + 1, hi - 1
        for ci in order:
            xr = xrpool.tile([P, 4, E], f32, tag="xr")
            if USE_HOST_GATHER:
                nc.sync.dma_start(xr[:], xg[:][:, 4 * ci:4 * ci + 4, :])
            else:
                nc.gpsimd.indirect_dma_start(
                    out=xr[:], out_offset=None, in_=emb[:],
                    in_offset=bass.IndirectOffsetOnAxis(
                        ap=idx_sb[:, 4 * ci:4 * ci + 4], axis=0),
                )
            nc.tensor.transpose(out=scr[:], in_=xr[0:32, 0, 0:32],
                                identity=ident[0:32, 0:32])
            for g in range(4):
                gb = 4 * ci + g
                for kt in range(KT):
                    pt = tppool.tile([P, P], f32, tag="tp")
                    nc.tensor.transpose(out=pt[:], in_=xr[:, g, kt * P:(kt + 1) * P],
                                        identity=ident[:])
                    nc.vector.tensor_copy(out=xT[:, kt, gb * P:(gb + 1) * P],
                                          in_=pt[:])
            s0 = 512 * ci
            for d in "fb":
                for m in range(M8):
                    pp = prpool.tile([P, 512], f32, tag="pr")
                    for kt in range(KT):
                        nc.tensor.matmul(out=pp[:], lhsT=w_sb[d][:, kt, m, :],
                                         rhs=xT[:, kt, s0:s0 + 512],
                                         start=(kt == 0), stop=(kt == KT - 1))
                    nc.vector.tensor_scalar_add(
                        out=xzT[d][:, m, s0:s0 + 512], in0=pp[:],
                        scalar1=b_sb[d][:, m:m + 1])

        # ---- recurrence ----
        cell = {d: spool.tile([P, KT, BS], f32, tag=f"c{d}", name=f"cell{d}") for d in "fb"}
        for d in "fb":
            nc.vector.memset(cell[d][:], 0.0)
        logits = {d: spool.tile([P, JT, NTAGS], f32, tag=f"lg{d}", name=f"logits{d}") for d in "fb"}
        hch = {"f": None, "b": None}
        hprev = {"f": None, "b": None}

        def step(d, tau):
            t = tau if d == "f" else (t_len - 1 - tau)
            sl = t % 8
            if tau % 8 == 0:
                hprev[d] = hch[d]
                hch[d] = hpool.tile([P, KT, 8 * BS], bf16, tag=f"h{d}", name=f"hch{d}")
            gates = gpool.tile([P, M8, BS], bf16, tag=f"g{d}")
            if tau == 0:
                nc.scalar.activation(gates[:, 0:6, :],
                                     xzT[d][:, 0:6, BS * t:BS * (t + 1)], SIG)
                nc.scalar.activation(gates[:, 6:8, :],
                                     xzT[d][:, 6:8, BS * t:BS * (t + 1)], TANH)
            else:
                tp = t + 1 if d == "b" else t - 1
                psl = tp % 8
                hsrc = hch[d] if tau % 8 != 0 else hprev[d]
                zp = zpool.tile([P, M8, BS], f32, tag=f"z{d}")
                for m in range(M8):
                    for kt in range(KT):
                        nc.tensor.matmul(
                            out=zp[:, m, :], lhsT=u_sb[d][:, kt, m, :],
                            rhs=hsrc[:, kt, BS * psl:BS * (psl + 1)],
                            start=(kt == 0), stop=(kt == KT - 1))
                nc.vector.tensor_tensor(out=zp[:], in0=zp[:],
                                        in1=xzT[d][:, :, BS * t:BS * (t + 1)],
                                        op=ADD)
                nc.scalar.activation(gates[:, 0:6, :], zp[:, 0:6, :], SIG)
                nc.scalar.activation(gates[:, 6:8, :], zp[:, 6:8, :], TANH)
            # cell update: c = f*c + i*g ; h = o*tanh(c)
            t1 = gpool.tile([P, KT, BS], bf16, tag=f"t1{d}")
            nc.vector.tensor_tensor(out=t1[:], in0=gates[:, 0:2, :],
                                    in1=gates[:, 6:8, :], op=MUL)
            nc.vector.tensor_tensor(out=cell[d][:], in0=gates[:, 2:4, :],
                                    in1=cell[d][:], op=MUL)
            nc.vector.tensor_tensor(out=cell[d][:], in0=cell[d][:], in1=t1[:],
                                    op=ADD)
            tct = gpool.tile([P, KT, BS], bf16, tag=f"tc{d}")
            nc.scalar.activation(tct[:], cell[d][:], TANH)
            nc.vector.tensor_tensor(out=hch[d][:, :, BS * sl:BS * (sl + 1)],
                                    in0=gates[:, 4:6, :], in1=tct[:], op=MUL)

        def dense(d, k):
            for jj in range(2):
                j = (2 * k + jj) if d == "f" else ((JT - 2) - 2 * k + jj)
                dp = dpool.tile([P, NTAGS], f32, tag="d")
                for kt in range(KT):
                    ktw = kt + (0 if d == "f" else KT)
                    nc.tensor.matmul(out=dp[:],
                                     lhsT=hch[d][:, kt, 128 * jj:128 * (jj + 1)],
                                     rhs=wd_sb[:, ktw, :],
                                     start=(kt == 0), stop=(kt == KT - 1))
                nc.vector.tensor_copy(out=logits[d][:, j, :], in_=dp[:])

        for tau in range(t_len):
            step("f", tau)
            step("b", tau)
            if tau % 8 == 7:
                dense("f", tau // 8)
                dense("b", tau // 8)

        # ---- bias + softmax (exp is safe unshifted: |logits| < ~6) ----
        nb = (JT + 7) // 8
        for bi in range(nb):
            j0 = 8 * bi
            jn = min(8, JT - j0)
            tmp = opool.tile([P, 8, NTAGS], f32, tag="sm")
            nc.vector.tensor_tensor(out=tmp[:, 0:jn, :],
                                    in0=logits["f"][:, j0:j0 + jn, :],
                                    in1=logits["b"][:, j0:j0 + jn, :], op=ADD)
            nc.vector.tensor_tensor(out=tmp[:, 0:jn, :], in0=tmp[:, 0:jn, :],
                                    in1=bd_sb[:, 0:jn, :],
                                    op=ADD)
            nc.scalar.activation(tmp[:, 0:jn, :], tmp[:, 0:jn, :], EXP)
            sm = opool.tile([P, 8, 1], f32, tag="smr")
            nc.vector.tensor_reduce(out=sm[:, 0:jn, :], in_=tmp[:, 0:jn, :],
                                    axis=mybir.AxisListType.X, op=ADD)
            rc = opool.tile([P, 8, 1], f32, tag="rc")
            nc.vector.reciprocal(out=rc[:, 0:jn, :], in_=sm[:, 0:jn, :])
            ost = opool.tile([P, 8, NTAGS], f32, tag="ost")
            nc.vector.tensor_tensor(out=ost[:, 0:jn, :], in0=tmp[:, 0:jn, :],
                                    in1=rc[:, 0:jn, :].to_broadcast([P, jn, NTAGS]),
                                    op=MUL)
            nc.sync.dma_start(out[:][:, j0:j0 + jn, :], ost[:, 0:jn, :])

    _legalize_waits(nc)
    return nc


# gate-column permutation: keras [i, f, g, o] -> ours [i, f, o, g]
def _gate_perm():
    return np.concatenate([np.arange(0, H), np.arange(H, 2 * H),
                           np.arange(3 * H, 4 * H), np.arange(2 * H, 3 * H)])


def marshal_weights(Wf, Uf, bf, Wb, Ub, bb, Wd, bd):
    import ml_dtypes
    perm = _gate_perm()
    def wmar(W):
        Wp = np.asarray(W, np.float32)[:, perm]
        return np.ascontiguousarray(
            Wp.reshape(KT, P, M8, P).transpose(1, 0, 2, 3)).astype(ml_dtypes.bfloat16)
    def bmar(b):
        bp = np.asarray(b, np.float32)[perm]
        return np.ascontiguousarray(bp.reshape(M8, P).T)
    wd = np.ascontiguousarray(
        np.asarray(Wd, np.float32).reshape(2 * KT, P, NTAGS)).astype(ml_dtypes.bfloat16)
    # [P, 2KT, NTAGS] with wd[p, kt, n] = Wd[kt*128+p, n]
    wd = np.ascontiguousarray(wd.transpose(1, 0, 2))
    bdt = np.ascontiguousarray(np.broadcast_to(np.tile(np.asarray(bd, np.float32), 8)[None, :], (P, 8 * NTAGS)))
    return {
        "w_f": wmar(Wf), "u_f": wmar(Uf), "b_f": bmar(bf),
        "w_b": wmar(Wb), "u_b": wmar(Ub), "b_b": bmar(bb),
        "wd": wd, "bd": bdt,
    }


def marshal_tokens(tokens_core, t_len=T):
    """tokens_core [BS, t_len] -> idx [128, t_len/4] int32 with
    idx[p, j] = tokens[p % 32, 4*j + p // 32]  (slot s = seq + 32*t)."""
    tk = np.asarray(tokens_core, np.int64)
    jt = BS * t_len // P
    p = np.arange(P)
    j = np.arange(jt)
    tt = 4 * j[None, :] + (p[:, None] // BS)
    return tk[(p[:, None] % BS), tt].astype(np.int32)


def unmarshal_out(out_core, t_len=T):
    """[128, JT, 17] slot-tile layout -> [BS, t_len, 17]."""
    slots = out_core.transpose(1, 0, 2).reshape(BS * t_len, NTAGS)
    return slots.reshape(t_len, BS, NTAGS).transpose(1, 0, 2)


def marshal_x(emb32, tokens_core, t_len=T):
    """Gather emb rows into the device slot layout [128, JT, E]."""
    idx = marshal_tokens(tokens_core, t_len)     # [128, JT] int32
    return np.ascontiguousarray(emb32[idx])      # [128, JT, E] f32


def kernel(tokens, emb, Wf, Uf, bf, Wb, Ub, bb, Wd, bd):
    from concourse.bass_utils import run_bass_kernel_spmd

    if "nc" not in _CACHE:
        _CACHE["nc"] = build_program()
    nc = _CACHE["nc"]

    weights = marshal_weights(Wf, Uf, bf, Wb, Ub, bb, Wd, bd)
    emb32 = np.ascontiguousarray(np.asarray(emb, np.float32))
    tokens = np.asarray(tokens)
    in_maps = []
    for c in range(NCORES):
        tk = tokens[BS * c:BS * (c + 1)]
        if USE_HOST_GATHER:
            m = {"xg": marshal_x(emb32, tk)}
        else:
            m = {"emb": emb32, "idx": marshal_tokens(tk)}
        m.update(weights)
        in_maps.append(m)
    res = run_bass_kernel_spmd(nc, in_maps, core_ids=list(range(NCORES)))
    outs = [unmarshal_out(res.results[c]["out"]) for c in range(NCORES)]
    return np.concatenate(outs, axis=0).astype(np.float32)


# revision 17
# speedup vs baseline: 4327.7607x; 4327.7607x over previous
"""BiLSTM tagger kernel for 8 Trainium2 NeuronCores.

Model (per reference): x = emb[tokens]; h_f = LSTM_f(x); h_b = LSTM_b(rev(x));
probs = softmax([h_f, h_b] @ Wd + bd).

Sharding: data-parallel over batch. Each of the 8 cores handles 32 sequences
and runs BOTH directions for them, so no cross-core communication is needed;
the host shards tokens and concatenates outputs.

Per-core layout ("transposed" LSTM): everything keeps the feature dim on SBUF
partitions and the 32 sequences on the free dim.  Token slot s = seq + 32*t.
 - gather: emb rows -> x_raw [128 part, slots/128, 256]  (indirect DMA)
 - PE-transpose -> xT [128 (E-slice), kt, slots] bf16
 - projection:  xzT[dir] [128 (4H-slice), m, slots] = W^T x + b   (bf16, bias
   folded, gates reordered host-side to [i, f, o, g] so sigmoid gates are
   contiguous)
 - recurrence (per direction, 128 steps): zT = U^T h in PSUM (16 matmuls,
   N=32), += xzT_t (DVE), sigmoid/tanh (ACT), cell update (DVE, fp32 cell),
   h written straight in matmul-rhs layout (no per-step transpose).
 - dense+softmax: logits accumulated incrementally per 8-step chunk from both
   directions, then bias + exp + normalize at the end.

Weights are marshalled host-side into the exact SBUF tile layouts (k-tile on
partitions) and cast to bf16; cell state and all accumulations stay fp32.
"""

import sys

import numpy as np

if "/opt/trn_rl_repo" not in sys.path:
    sys.path.insert(0, "/opt/trn_rl_repo")

V, E, T, H, NTAGS, B = 50000, 256, 128, 256, 17, 256
NCORES = 8
BS = B // NCORES            # sequences per core
P = 128
KT = E // P                 # 2 k-tiles for E and H
M8 = (4 * H) // P           # 8 m-tiles over the gate dim
# The SWDGE indirect-DMA (gather) path is unreliable in this environment
# (works after boot, breaks persistently after any device fault), so the
# embedding rows are gathered host-side into the slot layout and streamed
# to the device as a regular input.  Device work is otherwise identical.
USE_HOST_GATHER = True
SKEW = 1
CELL_BF16 = True

_CACHE = {}


def _legalize_waits(nc):
    """TRN2 hw instructions have one semaphore-wait slot; Tile can attach
    several.  Split extras onto same-engine NOPs placed just before."""
    import concourse.mybir as mybir

    for _, bbb in nc.bb_map.items():
        bb = bbb.bb
        new = []
        for inst in bb.instructions:
            si = inst.sync_info
            waits = list(si.on_wait) if (si and si.on_wait) else []
            if len(waits) > 1:
                for k, w in enumerate(waits[:-1]):
                    nop = mybir.InstNoOp(
                        name=f"{inst.name}_lw{k}",
                        engine=inst.engine,
                        sync_info=mybir.SyncInfo(on_wait=[w], on_update=[]),
                        bass_nofuse=True,
                    )
                    nc.register_instruction(nop)
                    new.append(nop)
                inst.sync_info = mybir.SyncInfo(
                    on_wait=[waits[-1]],
                    on_update=list(si.on_update) if si.on_update else [],
                )
            new.append(inst)
        bb.instructions = new


def build_program(t_len=T, vocab=V):
    """Build the per-core SPMD program.  t_len must be a multiple of 16."""
    from contextlib import ExitStack

    import concourse.bass as bass
    import concourse.mybir as mybir
    import concourse.tile as tile
    from concourse.masks import make_identity

    f32 = mybir.dt.float32
    bf16 = mybir.dt.bfloat16
    SIG = mybir.ActivationFunctionType.Sigmoid
    TANH = mybir.ActivationFunctionType.Tanh
    EXP = mybir.ActivationFunctionType.Exp
    MUL = mybir.AluOpType.mult
    ADD = mybir.AluOpType.add

    CDT = bf16 if CELL_BF16 else f32
    SLOTS = BS * t_len
    JT = SLOTS // P             # 128-slot tiles (= t_len/4)
    NCH = t_len // 16           # projection chunks of 512 slots
    NRCH = t_len // 8           # recurrence chunks of 8 steps

    nc = bass.Bass("TRN2", target_bir_lowering=False, debug=False)

    if USE_HOST_GATHER:
        xg = nc.dram_tensor("xg", [P, JT, E], f32, kind="ExternalInput")
    else:
        emb = nc.dram_tensor("emb", [vocab, E], f32, kind="ExternalInput")
        idx = nc.dram_tensor("idx", [P, JT], mybir.dt.int32, kind="ExternalInput")
    w_in = {d: nc.dram_tensor(f"w_{d}", [P, KT, M8, P], bf16, kind="ExternalInput")
            for d in "fb"}
    u_in = {d: nc.dram_tensor(f"u_{d}", [P, KT, M8, P], bf16, kind="ExternalInput")
            for d in "fb"}
    b_in = {d: nc.dram_tensor(f"b_{d}", [P, M8], f32, kind="ExternalInput")
            for d in "fb"}
    wd_in = nc.dram_tensor("wd", [P, 2 * KT, NTAGS], bf16, kind="ExternalInput")
    bd_in = nc.dram_tensor("bd", [P, 8 * NTAGS], f32, kind="ExternalInput")
    out = nc.dram_tensor("out", [P, JT, NTAGS], f32, kind="ExternalOutput")

    with tile.TileContext(nc) as tc, ExitStack() as ctx:
        cpool = ctx.enter_context(tc.tile_pool(name="const", bufs=1))
        xzpool = ctx.enter_context(tc.tile_pool(name="xz", bufs=1))
        xtpool = ctx.enter_context(tc.tile_pool(name="xt", bufs=1))
        xrpool = ctx.enter_context(tc.tile_pool(name="xr", bufs=2))
        gpool = ctx.enter_context(tc.tile_pool(name="g", bufs=2))
        hpool = ctx.enter_context(tc.tile_pool(name="h", bufs=2))
        spool = ctx.enter_context(tc.tile_pool(name="s", bufs=1))
        opool = ctx.enter_context(tc.tile_pool(name="o", bufs=2))
        tppool = ctx.enter_context(tc.tile_pool(name="tp", bufs=1, space="PSUM"))
        prpool = ctx.enter_context(tc.tile_pool(name="pr", bufs=2, space="PSUM"))
        zpool = ctx.enter_context(tc.tile_pool(name="z", bufs=2, space="PSUM"))
        dpool = ctx.enter_context(tc.tile_pool(name="d", bufs=1, space="PSUM"))

        # ---- constant loads ----
        if not USE_HOST_GATHER:
            idx_sb = cpool.tile([P, JT], mybir.dt.int32)
            nc.sync.dma_start(idx_sb[:], idx[:])
        ident = cpool.tile([P, P], f32)
        make_identity(nc, ident[:])
        ident_bf = cpool.tile([P, P], bf16)
        nc.vector.tensor_copy(ident_bf[:], ident[:])
        ones_row = cpool.tile([1, 512], f32)
        nc.vector.memset(ones_row[:], 1.0)
        w_sb, u_sb, b_sb = {}, {}, {}
        for d in "fb":
            w_sb[d] = cpool.tile([P, KT, M8, P], bf16, tag=f"w{d}", name=f"wsb{d}")
            nc.sync.dma_start(w_sb[d][:], w_in[d][:])
            u_sb[d] = cpool.tile([P, KT, M8, P], bf16, tag=f"u{d}", name=f"usb{d}")
            nc.sync.dma_start(u_sb[d][:], u_in[d][:])
            b_sb[d] = cpool.tile([P, M8], f32, tag=f"b{d}", name=f"bsb{d}")
            nc.sync.dma_start(b_sb[d][:], b_in[d][:])
        wd_sb = cpool.tile([P, 2 * KT, NTAGS], bf16)
        nc.sync.dma_start(wd_sb[:], wd_in[:])
        bd_sb = cpool.tile([P, 8, NTAGS], f32)
        nc.sync.dma_start(bd_sb[:], bd_in[:])

        xzT = {d: xzpool.tile([P, M8, SLOTS], bf16, tag=f"xz{d}", name=f"xzT{d}") for d in "fb"}
        xT = xtpool.tile([P, KT, SLOTS], bf16)

        # dense-psum bank doubles as PE-only scratch (disjoint column ranges):
        # scratch absorbs cross-engine waits so transpose matmuls (single hw
        # wait slot) never need two.
        dp_tile = dpool.tile([P, 64], f32)
        scr = dp_tile[0:32, 32:64]
        nc.tensor.transpose(out=scr, in_=ident[0:32, 0:32],
                            identity=ident[0:32, 0:32])

        # ---- gather + transpose + projection, chunk-pipelined ----
        # fwd consumes slots ascending, bwd descending: alternate chunk order.
        order = []
        lo, hi = 0, NCH - 1
        while lo <= hi:
            order.append(lo)
            if hi != lo:
                order.append(hi)
            lo, hi = lo + 1, hi - 1
        prelude_cm = tc.high_priority(offset=-1_000_000)
        prelude_cm.__enter__()
        for ci in order:
            xr = xrpool.tile([P, 4, E], f32, tag="xr")
            if USE_HOST_GATHER:
                nc.sync.dma_start(xr[:], xg[:][:, 4 * ci:4 * ci + 4, :])
            else:
                nc.gpsimd.indirect_dma_start(
                    out=xr[:], out_offset=None, in_=emb[:],
                    in_offset=bass.IndirectOffsetOnAxis(
                        ap=idx_sb[:, 4 * ci:4 * ci + 4], axis=0),
                )
            nc.tensor.transpose(out=scr, in_=xr[0:32, 0, 0:32],
                                identity=ident[0:32, 0:32])
            for g in range(4):
                gb = 4 * ci + g
                for kt in range(KT):
                    pt = tppool.tile([P, P], f32, tag="tp")
                    nc.tensor.transpose(out=pt[:], in_=xr[:, g, kt * P:(kt + 1) * P],
                                        identity=ident[:])
                    nc.scalar.copy(out=xT[:, kt, gb * P:(gb + 1) * P], in_=pt[:])
            s0 = 512 * ci
            for d in "fb":
                for m in range(M8):
                    pp = prpool.tile([P, 512], f32, tag="pr")
                    for kt in range(KT):
                        nc.tensor.matmul(out=pp[:], lhsT=w_sb[d][:, kt, m, :],
                                         rhs=xT[:, kt, s0:s0 + 512],
                                         start=(kt == 0), stop=(kt == KT - 1))
                    nc.vector.tensor_scalar_add(
                        out=xzT[d][:, m, s0:s0 + 512], in0=pp[:],
                        scalar1=b_sb[d][:, m:m + 1])

        prelude_cm.__exit__(None, None, None)

        # ---- recurrence ----
        cell = {d: spool.tile([P, KT, BS], CDT, tag=f"c{d}", name=f"cell{d}") for d in "fb"}
        for d in "fb":
            nc.vector.memset(cell[d][:], 0.0)
        logits = {d: spool.tile([P, JT, NTAGS], f32, tag=f"lg{d}", name=f"logits{d}") for d in "fb"}
        hch = {"f": None, "b": None}
        hprev = {"f": None, "b": None}

        last_sig = {"f": None, "b": None}

        def step(d, tau):
            t = tau if d == "f" else (t_len - 1 - tau)
            sl = t % 8
            if tau % 8 == 0:
                hprev[d] = hch[d]
                hch[d] = hpool.tile([P, KT, 8 * BS], bf16, tag=f"h{d}", name=f"hch{d}")
            gates = gpool.tile([P, M8, BS], bf16, tag=f"g{d}")
            if tau == 0:
                nc.scalar.activation(gates[:, 0:8, :],
                                     xzT[d][:, 0:8, BS * t:BS * (t + 1)], SIG)
            else:
                tp = t + 1 if d == "b" else t - 1
                psl = tp % 8
                hsrc = hch[d] if tau % 8 != 0 else hprev[d]
                zp = zpool.tile([P, M8, BS], f32, tag=f"z{d}")
                idmm = nc.tensor.matmul(
                    out=zp[:], lhsT=ident_bf[:],
                    rhs=xzT[d][:, :, BS * t:BS * (t + 1)],
                    start=True, stop=False)
                other = last_sig["b" if d == "f" else "f"]
                if SKEW and other is not None:
                    tile.add_dep_helper(other, idmm.ins, sync=(SKEW == 2),
                                        reason="chain skew")
                for m in range(M8):
                    for kt in range(KT):
                        nc.tensor.matmul(
                            out=zp[:, m, :], lhsT=u_sb[d][:, kt, m, :],
                            rhs=hsrc[:, kt, BS * psl:BS * (psl + 1)],
                            start=False, stop=(m == M8 - 1 and kt == KT - 1))
                last_sig[d] = nc.scalar.activation(gates[:, 0:8, :],
                                                   zp[:, 0:8, :], SIG).ins
            # cell update: c = f*c + i*g ; h = o*tanh(c)
            # g was computed as sigmoid(2*zg) (host pre-scales g columns x2):
            # tanh(zg) = 2*sigmoid(2*zg) - 1
            nc.vector.tensor_scalar(out=gates[:, 6:8, :], in0=gates[:, 6:8, :],
                                    scalar1=2.0, scalar2=1.0,
                                    op0=MUL, op1=mybir.AluOpType.subtract)
            t1 = gpool.tile([P, KT, BS], bf16, tag=f"t1{d}")
            nc.vector.tensor_tensor(out=t1[:], in0=gates[:, 0:2, :],
                                    in1=gates[:, 6:8, :], op=MUL)
            nc.vector.tensor_tensor(out=cell[d][:], in0=gates[:, 2:4, :],
                                    in1=cell[d][:], op=MUL)
            nc.vector.tensor_tensor(out=cell[d][:], in0=cell[d][:], in1=t1[:],
                                    op=ADD)
            tct = gpool.tile([P, KT, BS], bf16, tag=f"tc{d}")
            nc.scalar.activation(tct[:], cell[d][:], TANH)
            nc.vector.tensor_tensor(out=hch[d][:, :, BS * sl:BS * (sl + 1)],
                                    in0=gates[:, 4:6, :], in1=tct[:], op=MUL)

        def dense(d, k):
            for jj in range(2):
                j = (2 * k + jj) if d == "f" else ((JT - 2) - 2 * k + jj)
                dp = dp_tile[:, 0:NTAGS]
                for kt in range(KT):
                    ktw = kt + (0 if d == "f" else KT)
                    nc.tensor.matmul(out=dp,
                                     lhsT=hch[d][:, kt, 128 * jj:128 * (jj + 1)],
                                     rhs=wd_sb[:, ktw, :],
                                     start=(kt == 0), stop=(kt == KT - 1))
                nc.scalar.copy(out=logits[d][:, j, :], in_=dp)

        for tau in range(t_len):
            step("f", tau)
            step("b", tau)
            if tau % 8 == 7:
                dense("f", tau // 8)
                dense("b", tau // 8)

        # ---- bias + softmax (exp is safe unshifted: |logits| < ~6) ----
        nb = (JT + 7) // 8
        for bi in range(nb):
            j0 = 8 * bi
            jn = min(8, JT - j0)
            tmp = opool.tile([P, 8, NTAGS], f32, tag="sm")
            nc.vector.tensor_tensor(out=tmp[:, 0:jn, :],
                                    in0=logits["f"][:, j0:j0 + jn, :],
                                    in1=logits["b"][:, j0:j0 + jn, :], op=ADD)
            nc.vector.tensor_tensor(out=tmp[:, 0:jn, :], in0=tmp[:, 0:jn, :],
                                    in1=bd_sb[:, 0:jn, :],
                                    op=ADD)
            nc.scalar.activation(tmp[:, 0:jn, :], tmp[:, 0:jn, :], EXP)
            sm = opool.tile([P, 8, 1], f32, tag="smr")
            nc.vector.tensor_reduce(out=sm[:, 0:jn, :], in_=tmp[:, 0:jn, :],
                                    axis=mybir.AxisListType.X, op=ADD)
            rc = opool.tile([P, 8, 1], f32, tag="rc")
            nc.vector.reciprocal(out=rc[:, 0:jn, :], in_=sm[:, 0:jn, :])
            ost = opool.tile([P, 8, NTAGS], f32, tag="ost")
            nc.vector.tensor_tensor(out=ost[:, 0:jn, :], in0=tmp[:, 0:jn, :],
                                    in1=rc[:, 0:jn, :].to_broadcast([P, jn, NTAGS]),
                                    op=MUL)
            nc.sync.dma_start(out[:][:, j0:j0 + jn, :], ost[:, 0:jn, :])

    _legalize_waits(nc)
    return nc


# gate-column permutation: keras [i, f, g, o] -> ours [i, f, o, g]
def _gate_perm():
    return np.concatenate([np.arange(0, H), np.arange(H, 2 * H),
                           np.arange(3 * H, 4 * H), np.arange(2 * H, 3 * H)])


def marshal_weights(Wf, Uf, bf, Wb, Ub, bb, Wd, bd):
    import ml_dtypes
    perm = _gate_perm()
    gscale = np.ones(4 * H, np.float32)
    gscale[3 * H:] = 2.0     # g-gate columns (after perm they sit last)
    def wmar(W):
        Wp = np.asarray(W, np.float32)[:, perm] * gscale
        return np.ascontiguousarray(
            Wp.reshape(KT, P, M8, P).transpose(1, 0, 2, 3)).astype(ml_dtypes.bfloat16)
    def bmar(b):
        bp = np.asarray(b, np.float32)[perm] * gscale
        return np.ascontiguousarray(bp.reshape(M8, P).T)
    wd = np.ascontiguousarray(
        np.asarray(Wd, np.float32).reshape(2 * KT, P, NTAGS)).astype(ml_dtypes.bfloat16)
    # [P, 2KT, NTAGS] with wd[p, kt, n] = Wd[kt*128+p, n]
    wd = np.ascontiguousarray(wd.transpose(1, 0, 2))
    bdt = np.ascontiguousarray(np.broadcast_to(np.tile(np.asarray(bd, np.float32), 8)[None, :], (P, 8 * NTAGS)))
    return {
        "w_f": wmar(Wf), "u_f": wmar(Uf), "b_f": bmar(bf),
        "w_b": wmar(Wb), "u_b": wmar(Ub), "b_b": bmar(bb),
        "wd": wd, "bd": bdt,
    }


def marshal_tokens(tokens_core, t_len=T):
    """tokens_core [BS, t_len] -> idx [128, t_len/4] int32 with
    idx[p, j] = tokens[p % 32, 4*j + p // 32]  (slot s = seq + 32*t)."""
    tk = np.asarray(tokens_core, np.int64)
    jt = BS * t_len // P
    p = np.arange(P)
    j = np.arange(jt)
    tt = 4 * j[None, :] + (p[:, None] // BS)
    return tk[(p[:, None] % BS), tt].astype(np.int32)


def unmarshal_out(out_core, t_len=T):
    """[128, JT, 17] slot-tile layout -> [BS, t_len, 17]."""
    slots = out_core.transpose(1, 0, 2).reshape(BS * t_len, NTAGS)
    return slots.reshape(t_len, BS, NTAGS).transpose(1, 0, 2)


def marshal_x(emb32, tokens_core, t_len=T):
    """Gather emb rows into the device slot layout [128, JT, E]."""
    idx = marshal_tokens(tokens_core, t_len)     # [128, JT] int32
    return np.ascontiguousarray(emb32[idx])      # [128, JT, E] f32


def kernel(tokens, emb, Wf, Uf, bf, Wb, Ub, bb, Wd, bd):
    from concourse.bass_utils import run_bass_kernel_spmd

    if "nc" not in _CACHE:
        _CACHE["nc"] = build_program()
    nc = _CACHE["nc"]

    weights = marshal_weights(Wf, Uf, bf, Wb, Ub, bb, Wd, bd)
    emb32 = np.ascontiguousarray(np.asarray(emb, np.float32))
    tokens = np.asarray(tokens)
    in_maps = []
    for c in range(NCORES):
        tk = tokens[BS * c:BS * (c + 1)]
        if USE_HOST_GATHER:
            m = {"xg": marshal_x(emb32, tk)}
        else:
            m = {"emb": emb32, "idx": marshal_tokens(tk)}
        m.update(weights)
        in_maps.append(m)
    res = run_bass_kernel_spmd(nc, in_maps, core_ids=list(range(NCORES)))
    outs = [unmarshal_out(res.results[c]["out"]) for c in range(NCORES)]
    return np.concatenate(outs, axis=0).astype(np.float32)
